# revision 48
# baseline (speedup 1.0000x reference)
"""CapsuleLinear (dynamic routing) Trainium2 kernel.

Reference computes priors = einsum('oli,bni->bonl', W, x) (302MB) then runs 3
routing iterations. We never materialize priors; per routing iteration:
    probs[n,o]   = softmax_o(logits[n,o])              (exp on ACT, Z on DVE)
    s[o,i]       = sum_n probs[n,o] * x[n,i]           (PE matmul, contract n)
    v_raw[o,l]   = sum_i W[o,l,i] * s[o,i]             (DVE mul+reduce)
    factor[o]    = ||v_raw||/(1+||v_raw||^2)           (squash, per-partition)
    wv[o,i]      = factor[o] * sum_l W[o,l,i]*v_raw[o,l]   (DVE mul+reduce,
                                                        factor pulled out of
                                                        the reduce: squash runs
                                                        CONCURRENT with the wv
                                                        mul+reduce)
    logits[n,o] += sum_i x[n,i] * wv[o,i]              (PE matmul, contract i,
                                                        accumulates in PSUM)
Sharding: data-parallel over batch N=32 -> 4 batches per core on 8 cores.
Weight (64,32,32) replicated. No collectives.

v3 (latency rework; v2 was dependency-bound at 51% DVE utilization):
  - squash factor applied AFTER the wv reduce (it is per-o, i.e. a per-
    partition scalar): the sq/ns/ln/exp/recip chain overlaps the wv
    mul+reduce instead of preceding it. v itself is only materialized at r=2.
  - exp/zsum/recip/xr at per-BATCH granularity, and each batch's exp for the
    NEXT iteration is emitted right after that batch's delta matmuls
    (software pipelining across iterations - engine queues are in-order).
  - s PSUM->SBUF copies moved to the gpsimd queue (ACT was congested at the
    iteration boundary); the uniform 1/64 prob of iter 0 is folded into the
    ones lhsT (memset 1/64) so the copy is a plain copy every iteration.
  - x arrives as two DMAs on two queues (sync + vector).
All big DVE ops are bf16-in/bf16-out (2x packed mode, ~0.52ns/elem); reduces
are 1x at any dtype (measured) so v_raw/zsum dtype only matters downstream.
sqrt(ns) is computed as exp(0.5*ln(ns)) so the whole kernel uses one ACT
table set (natural_log_exp_and_others) - no 1.3us table switches.
"""

import os
import sys

for _p in ("/opt/trn_rl_repo",):
    if _p not in sys.path and os.path.isdir(_p):
        sys.path.insert(0, _p)

import numpy as np

import concourse.bacc as bacc
import concourse.bass as bass
import concourse.tile as tile
from concourse import mybir
from concourse.bass_utils import run_bass_kernel_spmd

N_TOT, N_CAPS, I_LEN = 32, 1152, 32
O_CAPS, L_LEN = 64, 32
NCORES = 8
B = N_TOT // NCORES  # 4 batches per core
C = N_CAPS // 128    # 9 chunks of 128 input capsules
PAIRS = B // 2       # 2 batches stacked on the 128 partitions
FP = mybir.dt.float32
BF = mybir.dt.bfloat16
Exp = mybir.ActivationFunctionType.Exp
Ln = mybir.ActivationFunctionType.Ln
X = mybir.AxisListType.X
MUL = mybir.AluOpType.mult


def build_nc():
    nc = bacc.Bacc("TRN2", target_bir_lowering=False, debug=True)
    x_nat_d = nc.dram_tensor("x_nat", [128, PAIRS, 2, C, I_LEN], BF, kind="ExternalInput")
    xt_d = nc.dram_tensor("xt", [I_LEN, B, C, 128], BF, kind="ExternalInput")
    w_li_d = nc.dram_tensor("w_li", [128, L_LEN, I_LEN], BF, kind="ExternalInput")
    m_d = nc.dram_tensor("m", [128, I_LEN, I_LEN], BF, kind="ExternalInput")
    ident_d = nc.dram_tensor("ident", [128, 128], FP, kind="ExternalInput")
    s0_d = nc.dram_tensor("s0", [128, PAIRS, I_LEN], BF, kind="ExternalInput")
    out_d = nc.dram_tensor("out", [PAIRS, 128, L_LEN], FP, kind="ExternalOutput")

    with tile.TileContext(nc) as tc, nc.allow_low_precision(
        reason="bf16 hot path; end-to-end rel err budget 2e-2"
    ):
        with (
            tc.tile_pool(name="main", bufs=1) as pool,
            tc.tile_pool(name="psum", bufs=1, space="PSUM") as psum,
        ):
            x_sb = pool.tile([128, PAIRS, 2, C, I_LEN], BF)
            xt_sb = pool.tile([I_LEN, B, C, 128], BF)
            wli_sb = pool.tile([128, L_LEN, I_LEN], BF)
            m_sb = pool.tile([128, I_LEN, I_LEN], BF)
            # PE transpose: out dtype must match identity, and fp32 rhs (wv)
            # requires fp32 identity -> the wv/wvT path stays fp32.
            ident = pool.tile([128, 128], FP)
            shift = pool.tile([128, 1], FP)
            pexp = pool.tile([128, PAIRS, 2, C, O_CAPS], BF)
            zsum = pool.tile([128, PAIRS, 2, C], BF)
            rinv = pool.tile([128, PAIRS, 2, C], FP)
            xr = pool.tile([128, PAIRS, 2, C, I_LEN], BF)
            s_sb = pool.tile([128, PAIRS, I_LEN], BF)
            s0_sb = pool.tile([128, PAIRS, I_LEN], BF)
            prod = pool.tile([128, PAIRS, L_LEN, I_LEN], BF)
            mprod = pool.tile([128, PAIRS, I_LEN, I_LEN], BF)
            nsprod = pool.tile([128, PAIRS, I_LEN], FP)
            v_rawf = pool.tile([128, PAIRS, L_LEN], FP)
            sqf = pool.tile([128, PAIRS, L_LEN], FP)
            ns = pool.tile([128, PAIRS], FP)
            lnns = pool.tile([128, PAIRS], FP)
            vnorm = pool.tile([128, PAIRS], FP)
            denom = pool.tile([128, PAIRS], FP)
            rden = pool.tile([128, PAIRS], FP)
            v = pool.tile([128, PAIRS, L_LEN], FP)
            wvraw = pool.tile([128, PAIRS, I_LEN], FP)
            wv = pool.tile([128, PAIRS, I_LEN], FP)
            wvt_sb = pool.tile([I_LEN, PAIRS, 128], BF)

            # logits PSUM, split into two 2-batch tiles so an iteration's
            # exp(b) only waits on its own half's delta matmuls. A matmul
            # with start=True lazily zeroes its whole bank, so emit start only
            # on the first chunk of each bank (r=0) and stop on the last.
            logits_ps = [
                psum.tile([128, 2, C, O_CAPS], FP, name=f"logits_ps{h}", tag=f"lg{h}")
                for h in range(2)
            ]
            # s (bytes 0..127) and wvT (bytes 512..1023) share a bank per pair;
            # the s -> v_raw -> wv -> wvT dependency chain orders their
            # lifetimes.
            u_ps = [
                psum.tile([128, 512], FP, name=f"u_ps{t}", tag=f"u_ps{t}")
                for t in range(PAIRS)
            ]
            s_ps = [u_ps[t][:, 0:I_LEN] for t in range(PAIRS)]
            wvt_ps = [u_ps[t][0:I_LEN, 128:256] for t in range(PAIRS)]

            dma = nc.sync
            # Queue assignment by first-use time (per-queue DMA latency is
            # ~2.5us and transfers serialize within a queue): iter 0 needs
            # only M+s0; wli only at iter 2's out-step; x isn't consumed
            # until iter 1's xr (~18us), xt by the first delta matmuls.
            dma.dma_start(out=m_sb[:], in_=m_d[:])
            nc.scalar.dma_start(out=s0_sb[:], in_=s0_d[:])
            nc.scalar.dma_start(out=wli_sb[:], in_=w_li_d[:])
            nc.gpsimd.dma_start(out=xt_sb[:], in_=xt_d[:])
            nc.gpsimd.dma_start(out=x_sb[:, 0], in_=x_nat_d[:, 0])
            nc.gpsimd.dma_start(out=x_sb[:, 1], in_=x_nat_d[:, 1])
            nc.gpsimd.dma_start(out=ident[:], in_=ident_d[:])
            nc.vector.memset(shift[:], -40.0)

            for r in range(3):
                # r=0 skips the s matmuls entirely: uniform probs make
                # s = sum_n(x)/64, which the host ships precomputed (s0) -
                # the out-step starts as soon as s0+w land, not after
                # x -> 36 matmuls -> copy.
                if r > 0:
                    for b in range(B):
                        t, b2 = divmod(b, 2)
                        # per-BATCH softmax chain: short refill legs pipeline
                        # best. xr on DVE: it's ready exactly in the
                        # iteration-boundary idle window, and recip->xr on one
                        # engine drops two cross-engine hops from the refill.
                        nc.vector.reduce_sum(
                            out=zsum[:, t, b2], in_=pexp[:, t, b2], axis=X
                        )
                        nc.vector.reciprocal(out=rinv[:, t, b2], in_=zsum[:, t, b2])
                        nc.vector.tensor_mul(
                            out=xr[:, t, b2],
                            in0=x_sb[:, t, b2],
                            in1=rinv[:, t, b2]
                            .unsqueeze(-1)
                            .broadcast_to((128, C, I_LEN)),
                        )
                        for c in range(C):
                            nc.tensor.matmul(
                                out=s_ps[t][b2 * 64 : (b2 + 1) * 64, :],
                                lhsT=pexp[:, t, b2, c, :],
                                rhs=xr[:, t, b2, c, :],
                                start=(c == 0),
                                stop=(c == C - 1),
                                tile_position=(0, 64 * b2),
                            )
                    # s PSUM->SBUF (+ fp32->bf16) copies: GPSIMD can't read
                    # PSUM, so these ride ACT.
                    for t in range(PAIRS):
                        nc.scalar.copy(out=s_sb[:, t, :], in_=s_ps[t][:])
                for t in range(PAIRS):
                    tsl = slice(t, t + 1)
                    # At r<2 only the AGREEMENT path is needed, and since
                    # squash's factor is a per-o scalar it commutes with the
                    # l-contraction:
                    #   wv = factor * (W^T W) @ s = factor * M @ s  (M on host)
                    #   ns = ||W @ s||^2 = s . (M @ s)              (tiny dot)
                    # so the whole out-step (mul+reduce+square, ~4us/iter on
                    # DVE) only runs at r=2 where v itself is the output.
                    s_cur = (s0_sb if r == 0 else s_sb)[:, t, :]
                    if r == 2:
                        nc.vector.tensor_mul(
                            out=prod[:, t],
                            in0=wli_sb[:],
                            in1=s_cur.unsqueeze(1).broadcast_to((128, L_LEN, I_LEN)),
                        )
                        nc.vector.reduce_sum(
                            out=v_rawf[:, t, :], in_=prod[:, t], axis=X
                        )
                        nc.vector.tensor_mul(
                            out=sqf[:, t], in0=v_rawf[:, t], in1=v_rawf[:, t]
                        )
                        nc.vector.reduce_sum(
                            out=ns[:, tsl], in_=sqf[:, t].unsqueeze(1), axis=X
                        )
                    else:
                        nc.vector.tensor_mul(
                            out=mprod[:, t],
                            in0=m_sb[:],
                            in1=s_cur.unsqueeze(1).broadcast_to((128, I_LEN, I_LEN)),
                        )
                        nc.vector.reduce_sum(
                            out=wvraw[:, t, :], in_=mprod[:, t], axis=X
                        )
                        nc.vector.tensor_mul(
                            out=nsprod[:, t], in0=wvraw[:, t, :], in1=s_cur
                        )
                        nc.vector.reduce_sum(
                            out=ns[:, tsl], in_=nsprod[:, t].unsqueeze(1), axis=X
                        )
                    nc.vector.tensor_scalar_add(
                        out=denom[:, tsl], in0=ns[:, tsl], scalar1=1.0
                    )
                    nc.vector.reciprocal(out=rden[:, tsl], in_=denom[:, tsl])
                    nc.scalar.activation(out=lnns[:, tsl], in_=ns[:, tsl], func=Ln)
                    nc.scalar.activation(
                        out=vnorm[:, tsl], in_=lnns[:, tsl], func=Exp, scale=0.5
                    )
                    if r == 2:
                        # v = (v_raw * ||v||) / (1+||v||^2)
                        nc.vector.tensor_scalar(
                            out=v[:, t],
                            in0=v_rawf[:, t],
                            scalar1=vnorm[:, tsl],
                            scalar2=rden[:, tsl],
                            op0=MUL,
                            op1=MUL,
                        )
                        dma.dma_start(out=out_d[t], in_=v[:, t, :])
                        continue
                    nc.vector.tensor_scalar(
                        out=wv[:, t, :],
                        in0=wvraw[:, t, :],
                        scalar1=vnorm[:, tsl],
                        scalar2=rden[:, tsl],
                        op0=MUL,
                        op1=MUL,
                    )
                    nc.tensor.transpose(
                        out=wvt_ps[t][:], in_=wv[:, t, :], identity=ident[:]
                    )
                    # logits[n,o] += sum_i x[n,i] * wv[o,i], then this pair's
                    # exps for the NEXT iteration right behind the chunks.
                    # The wvT copy is split per half so b2=0's matmuls start
                    # one copy earlier.
                    # r0: one start/stop per 2KB psum bank (8 chunks per bank).
                    # r1: accumulate onto surviving has_written bits; the sim's
                    # group bookkeeping can't express re-opening, so skip it.
                    # (PSUM banks span the two b2 halves, so the exps can only
                    # go after all 18 chunks close their accumulation groups.)
                    for b2 in range(2):
                        nc.scalar.copy(
                            out=wvt_sb[:, t, b2 * 64 : (b2 + 1) * 64],
                            in_=wvt_ps[t][:, b2 * 64 : (b2 + 1) * 64],
                        )
                        for c in range(C):
                            k = b2 * C + c
                            nc.tensor.matmul(
                                out=logits_ps[t][:, b2, c, :],
                                lhsT=xt_sb[:, 2 * t + b2, c, :],
                                rhs=wvt_sb[:, t, b2 * 64 : (b2 + 1) * 64],
                                start=(r == 0 and k % 8 == 0),
                                stop=(r == 0 and (k % 8 == 7 or k == 2 * C - 1)),
                                skip_group_check=(r == 1),
                            )
                    for b2 in range(2):
                        nc.scalar.activation(
                            out=pexp[:, t, b2],
                            in_=logits_ps[t][:, b2],
                            func=Exp,
                            bias=shift[:],
                        )
    return nc


_NC = None


def get_nc():
    global _NC
    if _NC is None:
        _NC = build_nc()
    return _NC


def to_bf16(a):
    import ml_dtypes

    return a.astype(ml_dtypes.bfloat16)


def make_in_maps(x, weight):
    x = np.ascontiguousarray(x, dtype=np.float32)
    w = np.ascontiguousarray(weight, dtype=np.float32)
    w_li = to_bf16(np.tile(w.reshape(O_CAPS, L_LEN, I_LEN), (2, 1, 1)))
    # M[o] = W[o]^T W[o]: the agreement step becomes wv = factor * M @ s and
    # ns = s . (M @ s), so iterations 0/1 never materialize v_raw on-chip.
    m = np.einsum("oli,olj->oij", w, w)
    m_rep = to_bf16(np.tile(m, (2, 1, 1)))
    ident = np.eye(128, dtype=np.float32)
    in_maps = []
    for core in range(NCORES):
        xs = x[core * B : (core + 1) * B]  # [B, 1152, 32]
        xc = xs.reshape(B, C, 128, I_LEN)
        x_nat = np.ascontiguousarray(xc.transpose(2, 0, 1, 3)).reshape(
            128, PAIRS, 2, C, I_LEN
        )
        xt = np.ascontiguousarray(xc.transpose(3, 0, 1, 2))  # [32, B, C, 128]
        # iter-0 probs are uniform -> s = sum_n(x)/64, same for every o:
        # ship it precomputed, replicated across each batch's 64 partitions.
        s0b = xs.sum(axis=1) / 64.0  # [B, 32]
        s0 = np.ascontiguousarray(
            s0b.reshape(PAIRS, 2, 1, I_LEN)  # [t, b2, o, i]
            .repeat(O_CAPS, axis=2)
            .transpose(1, 2, 0, 3)  # [b2, o, t, i]
            .reshape(128, PAIRS, I_LEN)
        )
        in_maps.append(
            {
                "x_nat": to_bf16(x_nat),
                "xt": to_bf16(xt),
                "w_li": w_li,
                "m": m_rep,
                "ident": ident,
                "s0": to_bf16(s0),
            }
        )
    return in_maps


def assemble(results):
    outs = []
    for core in range(NCORES):
        o = results[core]["out"]  # [PAIRS, 128, 32] -> [4, 64, 32]
        outs.append(np.asarray(o, dtype=np.float32).reshape(B, O_CAPS, L_LEN))
    return np.concatenate(outs, axis=0)


def _pin_act_table_set(nc):
    """Make Exp and Ln resolve to the one table set containing both
    (natural_log_exp_and_others), so the whole kernel runs on a single
    ACT table load instead of thrashing 1.3us loads between exp/ln sets.
    Mutates the cached dict in place; set indices stay aligned with
    act_info.json."""
    from concourse.hw_specs import get_activation_tables

    tabs = get_activation_tables(nc.m.arch)
    for name, funcs in tabs.items():
        if name != "natural_log_exp_and_others":
            funcs.discard(Exp)
            funcs.discard(Ln)
            funcs.discard(mybir.ActivationFunctionType.Square)
            funcs.discard(mybir.ActivationFunctionType.Copy)
            funcs.discard(mybir.ActivationFunctionType.Identity)


def run(x, weight, trace=False):
    nc = get_nc()
    if not nc.is_finalized():
        _pin_act_table_set(nc)
        nc.finalize()  # run Bacc lowering passes (wait splitting, reg alloc)
    res = run_bass_kernel_spmd(nc, make_in_maps(x, weight), list(range(NCORES)), trace=trace)
    return assemble(res.results), res


def kernel(x, weight):
    out, _ = run(x, weight)
    return out


# revision 50
# speedup vs baseline: 1.0062x; 1.0062x over previous
"""CapsuleLinear (dynamic routing) Trainium2 kernel.

Reference computes priors = einsum('oli,bni->bonl', W, x) (302MB) then runs 3
routing iterations. We never materialize priors; per routing iteration:
    probs[n,o]   = softmax_o(logits[n,o])              (exp on ACT, Z on DVE)
    s[o,i]       = sum_n probs[n,o] * x[n,i]           (PE matmul, contract n)
    v_raw[o,l]   = sum_i W[o,l,i] * s[o,i]             (DVE mul+reduce)
    factor[o]    = ||v_raw||/(1+||v_raw||^2)           (squash, per-partition)
    wv[o,i]      = factor[o] * sum_l W[o,l,i]*v_raw[o,l]   (DVE mul+reduce,
                                                        factor pulled out of
                                                        the reduce: squash runs
                                                        CONCURRENT with the wv
                                                        mul+reduce)
    logits[n,o] += sum_i x[n,i] * wv[o,i]              (PE matmul, contract i,
                                                        accumulates in PSUM)
Sharding: data-parallel over batch N=32 -> 4 batches per core on 8 cores.
Weight (64,32,32) replicated. No collectives.

v3 (latency rework; v2 was dependency-bound at 51% DVE utilization):
  - squash factor applied AFTER the wv reduce (it is per-o, i.e. a per-
    partition scalar): the sq/ns/ln/exp/recip chain overlaps the wv
    mul+reduce instead of preceding it. v itself is only materialized at r=2.
  - exp/zsum/recip/xr at per-BATCH granularity, and each batch's exp for the
    NEXT iteration is emitted right after that batch's delta matmuls
    (software pipelining across iterations - engine queues are in-order).
  - s PSUM->SBUF copies moved to the gpsimd queue (ACT was congested at the
    iteration boundary); the uniform 1/64 prob of iter 0 is folded into the
    ones lhsT (memset 1/64) so the copy is a plain copy every iteration.
  - x arrives as two DMAs on two queues (sync + vector).
All big DVE ops are bf16-in/bf16-out (2x packed mode, ~0.52ns/elem); reduces
are 1x at any dtype (measured) so v_raw/zsum dtype only matters downstream.
sqrt(ns) is computed as exp(0.5*ln(ns)) so the whole kernel uses one ACT
table set (natural_log_exp_and_others) - no 1.3us table switches.
"""

import os
import sys

for _p in ("/opt/trn_rl_repo",):
    if _p not in sys.path and os.path.isdir(_p):
        sys.path.insert(0, _p)

import numpy as np

import concourse.bacc as bacc
import concourse.bass as bass
import concourse.tile as tile
from concourse import mybir
from concourse.bass_utils import run_bass_kernel_spmd

N_TOT, N_CAPS, I_LEN = 32, 1152, 32
O_CAPS, L_LEN = 64, 32
NCORES = 8
B = N_TOT // NCORES  # 4 batches per core
C = N_CAPS // 128    # 9 chunks of 128 input capsules
PAIRS = B // 2       # 2 batches stacked on the 128 partitions
FP = mybir.dt.float32
BF = mybir.dt.bfloat16
Exp = mybir.ActivationFunctionType.Exp
Ln = mybir.ActivationFunctionType.Ln
X = mybir.AxisListType.X
MUL = mybir.AluOpType.mult


def build_nc():
    nc = bacc.Bacc("TRN2", target_bir_lowering=False, debug=True)
    x_nat_d = nc.dram_tensor("x_nat", [128, PAIRS, 2, C, I_LEN], BF, kind="ExternalInput")
    xt_d = nc.dram_tensor("xt", [I_LEN, B, C, 128], BF, kind="ExternalInput")
    w_li_d = nc.dram_tensor("w_li", [128, L_LEN, I_LEN], BF, kind="ExternalInput")
    m_d = nc.dram_tensor("m", [128, I_LEN, I_LEN], BF, kind="ExternalInput")
    ident_d = nc.dram_tensor("ident", [128, 128], FP, kind="ExternalInput")
    s0_d = nc.dram_tensor("s0", [128, PAIRS, I_LEN], BF, kind="ExternalInput")
    out_d = nc.dram_tensor("out", [PAIRS, 128, L_LEN], FP, kind="ExternalOutput")

    with tile.TileContext(nc) as tc, nc.allow_low_precision(
        reason="bf16 hot path; end-to-end rel err budget 2e-2"
    ):
        with (
            tc.tile_pool(name="main", bufs=1) as pool,
            tc.tile_pool(name="psum", bufs=1, space="PSUM") as psum,
        ):
            x_sb = pool.tile([128, PAIRS, 2, C, I_LEN], BF)
            xt_sb = pool.tile([I_LEN, B, C, 128], BF)
            wli_sb = pool.tile([128, L_LEN, I_LEN], BF)
            m_sb = pool.tile([128, I_LEN, I_LEN], BF)
            # PE transpose: out dtype must match identity, and fp32 rhs (wv)
            # requires fp32 identity -> the wv/wvT path stays fp32.
            ident = pool.tile([128, 128], FP)
            shift = pool.tile([128, 1], FP)
            pexp = pool.tile([128, PAIRS, 2, C, O_CAPS], BF)
            zsum = pool.tile([128, PAIRS, 2, C], BF)
            rinv = pool.tile([128, PAIRS, 2, C], FP)
            xr = pool.tile([128, PAIRS, 2, C, I_LEN], BF)
            s_sb = pool.tile([128, PAIRS, I_LEN], BF)
            s0_sb = pool.tile([128, PAIRS, I_LEN], BF)
            prod = pool.tile([128, PAIRS, L_LEN, I_LEN], BF)
            mprod = pool.tile([128, PAIRS, I_LEN, I_LEN], BF)
            nsprod = pool.tile([128, PAIRS, I_LEN], FP)
            v_rawf = pool.tile([128, PAIRS, L_LEN], FP)
            sqf = pool.tile([128, PAIRS, L_LEN], FP)
            ns = pool.tile([128, PAIRS], FP)
            lnns = pool.tile([128, PAIRS], FP)
            vnorm = pool.tile([128, PAIRS], FP)
            denom = pool.tile([128, PAIRS], FP)
            rden = pool.tile([128, PAIRS], FP)
            v = pool.tile([128, PAIRS, L_LEN], FP)
            wvraw = pool.tile([128, PAIRS, I_LEN], FP)
            wv = pool.tile([128, PAIRS, I_LEN], FP)
            wvt_sb = pool.tile([I_LEN, PAIRS, 128], BF)

            # logits PSUM, split into two 2-batch tiles so an iteration's
            # exp(b) only waits on its own half's delta matmuls. A matmul
            # with start=True lazily zeroes its whole bank, so emit start only
            # on the first chunk of each bank (r=0) and stop on the last.
            logits_ps = [
                psum.tile([128, 2, C, O_CAPS], FP, name=f"logits_ps{h}", tag=f"lg{h}")
                for h in range(2)
            ]
            # s (bytes 0..127) and wvT (bytes 512..1023) share a bank per pair;
            # the s -> v_raw -> wv -> wvT dependency chain orders their
            # lifetimes.
            u_ps = [
                psum.tile([128, 512], FP, name=f"u_ps{t}", tag=f"u_ps{t}")
                for t in range(PAIRS)
            ]
            s_ps = [u_ps[t][:, 0:I_LEN] for t in range(PAIRS)]
            wvt_ps = [u_ps[t][0:I_LEN, 128:256] for t in range(PAIRS)]

            dma = nc.sync
            # Queue assignment by first-use time (per-queue DMA latency is
            # ~2.5us and transfers serialize within a queue): iter 0 needs
            # only M+s0; wli only at iter 2's out-step; x isn't consumed
            # until iter 1's xr (~18us), xt by the first delta matmuls.
            dma.dma_start(out=m_sb[:], in_=m_d[:])
            nc.scalar.dma_start(out=s0_sb[:], in_=s0_d[:])
            nc.scalar.dma_start(out=wli_sb[:], in_=w_li_d[:])
            nc.gpsimd.dma_start(out=xt_sb[:], in_=xt_d[:])
            nc.gpsimd.dma_start(out=x_sb[:, 0], in_=x_nat_d[:, 0])
            nc.gpsimd.dma_start(out=x_sb[:, 1], in_=x_nat_d[:, 1])
            nc.gpsimd.dma_start(out=ident[:], in_=ident_d[:])
            nc.vector.memset(shift[:], -40.0)

            for r in range(3):
                # r=0 skips the s matmuls entirely: uniform probs make
                # s = sum_n(x)/64, which the host ships precomputed (s0) -
                # the out-step starts as soon as s0+w land, not after
                # x -> 36 matmuls -> copy.
                if r > 0:
                    for b in range(B):
                        t, b2 = divmod(b, 2)
                        # per-BATCH softmax chain: short refill legs pipeline
                        # best (per-pair fronts, xr-on-DVE, and gpsimd squash
                        # offloads all measured slower).
                        nc.vector.reduce_sum(
                            out=zsum[:, t, b2], in_=pexp[:, t, b2], axis=X
                        )
                        nc.vector.reciprocal(out=rinv[:, t, b2], in_=zsum[:, t, b2])
                        nc.gpsimd.tensor_mul(
                            out=xr[:, t, b2],
                            in0=x_sb[:, t, b2],
                            in1=rinv[:, t, b2]
                            .unsqueeze(-1)
                            .broadcast_to((128, C, I_LEN)),
                        )
                        for c in range(C):
                            nc.tensor.matmul(
                                out=s_ps[t][b2 * 64 : (b2 + 1) * 64, :],
                                lhsT=pexp[:, t, b2, c, :],
                                rhs=xr[:, t, b2, c, :],
                                start=(c == 0),
                                stop=(c == C - 1),
                                tile_position=(0, 64 * b2),
                            )
                    # s PSUM->SBUF (+ fp32->bf16) copies: GPSIMD can't read
                    # PSUM, so these ride ACT.
                    for t in range(PAIRS):
                        nc.scalar.copy(out=s_sb[:, t, :], in_=s_ps[t][:])
                for t in range(PAIRS):
                    tsl = slice(t, t + 1)
                    # At r<2 only the AGREEMENT path is needed, and since
                    # squash's factor is a per-o scalar it commutes with the
                    # l-contraction:
                    #   wv = factor * (W^T W) @ s = factor * M @ s  (M on host)
                    #   ns = ||W @ s||^2 = s . (M @ s)              (tiny dot)
                    # so the whole out-step (mul+reduce+square, ~4us/iter on
                    # DVE) only runs at r=2 where v itself is the output.
                    s_cur = (s0_sb if r == 0 else s_sb)[:, t, :]
                    if r == 2:
                        nc.vector.tensor_mul(
                            out=prod[:, t],
                            in0=wli_sb[:],
                            in1=s_cur.unsqueeze(1).broadcast_to((128, L_LEN, I_LEN)),
                        )
                        nc.vector.reduce_sum(
                            out=v_rawf[:, t, :], in_=prod[:, t], axis=X
                        )
                        nc.vector.tensor_mul(
                            out=sqf[:, t], in0=v_rawf[:, t], in1=v_rawf[:, t]
                        )
                        nc.vector.reduce_sum(
                            out=ns[:, tsl], in_=sqf[:, t].unsqueeze(1), axis=X
                        )
                    else:
                        nc.vector.tensor_mul(
                            out=mprod[:, t],
                            in0=m_sb[:],
                            in1=s_cur.unsqueeze(1).broadcast_to((128, I_LEN, I_LEN)),
                        )
                        nc.vector.reduce_sum(
                            out=wvraw[:, t, :], in_=mprod[:, t], axis=X
                        )
                        nc.vector.tensor_mul(
                            out=nsprod[:, t], in0=wvraw[:, t, :], in1=s_cur
                        )
                        nc.vector.reduce_sum(
                            out=ns[:, tsl], in_=nsprod[:, t].unsqueeze(1), axis=X
                        )
                    nc.vector.tensor_scalar_add(
                        out=denom[:, tsl], in0=ns[:, tsl], scalar1=1.0
                    )
                    nc.vector.reciprocal(out=rden[:, tsl], in_=denom[:, tsl])
                    nc.scalar.activation(out=lnns[:, tsl], in_=ns[:, tsl], func=Ln)
                    nc.scalar.activation(
                        out=vnorm[:, tsl], in_=lnns[:, tsl], func=Exp, scale=0.5
                    )
                    if r == 2:
                        # v = (v_raw * ||v||) / (1+||v||^2)
                        nc.vector.tensor_scalar(
                            out=v[:, t],
                            in0=v_rawf[:, t],
                            scalar1=vnorm[:, tsl],
                            scalar2=rden[:, tsl],
                            op0=MUL,
                            op1=MUL,
                        )
                        dma.dma_start(out=out_d[t], in_=v[:, t, :])
                        continue
                    nc.vector.tensor_scalar(
                        out=wv[:, t, :],
                        in0=wvraw[:, t, :],
                        scalar1=vnorm[:, tsl],
                        scalar2=rden[:, tsl],
                        op0=MUL,
                        op1=MUL,
                    )
                    nc.tensor.transpose(
                        out=wvt_ps[t][:], in_=wv[:, t, :], identity=ident[:]
                    )
                    nc.scalar.copy(out=wvt_sb[:, t, :], in_=wvt_ps[t][:])
                    # logits[n,o] += sum_i x[n,i] * wv[o,i], then this pair's
                    # exps for the NEXT iteration right behind the chunks.
                    # r0: one start/stop per 2KB psum bank (8 chunks per bank).
                    # r1: accumulate onto surviving has_written bits; the sim's
                    # group bookkeeping can't express re-opening, so skip it.
                    # (PSUM banks span the two b2 halves, so the exps can only
                    # go after all 18 chunks close their accumulation groups.)
                    for b2 in range(2):
                        for c in range(C):
                            k = b2 * C + c
                            nc.tensor.matmul(
                                out=logits_ps[t][:, b2, c, :],
                                lhsT=xt_sb[:, 2 * t + b2, c, :],
                                rhs=wvt_sb[:, t, b2 * 64 : (b2 + 1) * 64],
                                start=(r == 0 and k % 8 == 0),
                                stop=(r == 0 and (k % 8 == 7 or k == 2 * C - 1)),
                                skip_group_check=(r == 1),
                            )
                    for b2 in range(2):
                        nc.scalar.activation(
                            out=pexp[:, t, b2],
                            in_=logits_ps[t][:, b2],
                            func=Exp,
                            bias=shift[:],
                        )
    return nc


_NC = None


def get_nc():
    global _NC
    if _NC is None:
        _NC = build_nc()
    return _NC


def to_bf16(a):
    import ml_dtypes

    return a.astype(ml_dtypes.bfloat16)


def make_in_maps(x, weight):
    x = np.ascontiguousarray(x, dtype=np.float32)
    w = np.ascontiguousarray(weight, dtype=np.float32)
    w_li = to_bf16(np.tile(w.reshape(O_CAPS, L_LEN, I_LEN), (2, 1, 1)))
    # M[o] = W[o]^T W[o]: the agreement step becomes wv = factor * M @ s and
    # ns = s . (M @ s), so iterations 0/1 never materialize v_raw on-chip.
    m = np.einsum("oli,olj->oij", w, w)
    m_rep = to_bf16(np.tile(m, (2, 1, 1)))
    ident = np.eye(128, dtype=np.float32)
    in_maps = []
    for core in range(NCORES):
        xs = x[core * B : (core + 1) * B]  # [B, 1152, 32]
        xc = xs.reshape(B, C, 128, I_LEN)
        x_nat = np.ascontiguousarray(xc.transpose(2, 0, 1, 3)).reshape(
            128, PAIRS, 2, C, I_LEN
        )
        xt = np.ascontiguousarray(xc.transpose(3, 0, 1, 2))  # [32, B, C, 128]
        # iter-0 probs are uniform -> s = sum_n(x)/64, same for every o:
        # ship it precomputed, replicated across each batch's 64 partitions.
        s0b = xs.sum(axis=1) / 64.0  # [B, 32]
        s0 = np.ascontiguousarray(
            s0b.reshape(PAIRS, 2, 1, I_LEN)  # [t, b2, o, i]
            .repeat(O_CAPS, axis=2)
            .transpose(1, 2, 0, 3)  # [b2, o, t, i]
            .reshape(128, PAIRS, I_LEN)
        )
        in_maps.append(
            {
                "x_nat": to_bf16(x_nat),
                "xt": to_bf16(xt),
                "w_li": w_li,
                "m": m_rep,
                "ident": ident,
                "s0": to_bf16(s0),
            }
        )
    return in_maps


def assemble(results):
    outs = []
    for core in range(NCORES):
        o = results[core]["out"]  # [PAIRS, 128, 32] -> [4, 64, 32]
        outs.append(np.asarray(o, dtype=np.float32).reshape(B, O_CAPS, L_LEN))
    return np.concatenate(outs, axis=0)


def _pin_act_table_set(nc):
    """Make Exp and Ln resolve to the one table set containing both
    (natural_log_exp_and_others), so the whole kernel runs on a single
    ACT table load instead of thrashing 1.3us loads between exp/ln sets.
    Mutates the cached dict in place; set indices stay aligned with
    act_info.json."""
    from concourse.hw_specs import get_activation_tables

    tabs = get_activation_tables(nc.m.arch)
    for name, funcs in tabs.items():
        if name != "natural_log_exp_and_others":
            funcs.discard(Exp)
            funcs.discard(Ln)
            funcs.discard(mybir.ActivationFunctionType.Square)
            funcs.discard(mybir.ActivationFunctionType.Copy)
            funcs.discard(mybir.ActivationFunctionType.Identity)


def run(x, weight, trace=False):
    nc = get_nc()
    if not nc.is_finalized():
        _pin_act_table_set(nc)
        nc.finalize()  # run Bacc lowering passes (wait splitting, reg alloc)
    res = run_bass_kernel_spmd(nc, make_in_maps(x, weight), list(range(NCORES)), trace=trace)
    return assemble(res.results), res


def kernel(x, weight):
    out, _ = run(x, weight)
    return out
